# revision 34
# baseline (speedup 1.0000x reference)
"""Trainium2 Bass kernel for nn_MultiHeadAttention_89232240541956.

Computes, for B=8, S=4096, H=1024, ATTN=1024, EXT=1152:
    x_ext = [h | broadcast(g) | l]                       [B, S, 1152]
    q = relu(x_ext @ Wq + bq); k = relu(x_ext @ Wk + bk) [B, S, 1024]
    scores = sum(q * k, -1) / 32, masked to -1e9 where mask == 1

Sharding: data-parallel over batch — core b owns batch b.

Key transformations:
  - v (Wv, bv) is dead code in the reference's early-return path — skipped.
  - g @ Wq[1024:1088] is constant over seq for a batch — folded into the
    bias on the host, so the device contracts over 1024 (h) + 64 (l) only.
  - Bias folded into the matmul as one extra contraction row (ones-row in
    x^T against a bias-row in W).
  - Host pre-transposes to x^T so the contraction dim lands on SBUF
    partitions with no on-device transposes; all DMAs are 2-D contiguous.
  - Mixed precision: NFP8 paired chunks (256 contraction dims each) run in
    fp8 E4M3 with DoubleRow (2 MACs/cell/cycle, halves the matmul count
    for those dims); the rest in bf16. x is pre-scaled by 16 and W by 64
    (lossless exponent shifts) to sit in E4M3's normal range; the 2^10
    product scale and 1/sqrt(1024) are divided out in the epilogue.
    Measured on the actual (seed-0) data vs the fp32 reference:
    NFP8=0: 1.7e-3 max rel err @ ~273us; NFP8=2: 1.6e-2 @ ~217us (ships);
    NFP8=4: 2.2e-2 @ ~163us. Max abs err at NFP8=2 is 0.025 against a
    score scale of ~1.7 mean / ~4.5 absmax (masked entries exact).
  - Epilogue: ScalarE relu (PSUM->SBUF bf16), DVE multiply, ScalarE
    Copy-with-accum row reduction; masking on host at unshard. The last
    tile runs q/k as separate matmul passes with a split-half epilogue to
    shorten the kernel tail.
  - PSUM: q/k tiles 2 banks each, double-buffered = all 8 banks; N=512
    per matmul (one fp32 bank) with accumulating chunk groups.
"""

import numpy as np
import ml_dtypes

B, S, H, LOC = 8, 4096, 1024, 64
ATTN = 1024
KL = LOC + 1              # 65 rows: l | ones/bias
SBLK = 512                # seq columns per DMA block
NBLK = S // SBLK          # 8
NT = SBLK // 128          # 4 seq tiles (128 tokens) per block
NCOL = S // 128           # 32 output columns

BF16 = ml_dtypes.bfloat16

# Number of paired fp8 chunks (256 h-dims each); remaining h-dims in bf16.
NFP8 = 2
XSCALE = 16.0
WSCALE = 64.0

_CACHE = {}


def _build_nc(nfp8=NFP8):
    import concourse.bass as bass
    import concourse.mybir as mybir
    import concourse.tile as tile
    from concourse import bacc

    dt = mybir.dt
    nbh = (H - 256 * nfp8) // 128         # bf16 h chunks
    scaled = nfp8 > 0
    nc = bacc.Bacc(None, target_bir_lowering=False)

    xh8_d = wq8_d = wk8_d = None
    xhb_d = wqb_d = wkb_d = None
    if nfp8:
        # paired layout: [c, blk, p, j, s] rows flattened; partition p of
        # chunk c holds contraction rows c*256 + 2p + j for j in {0,1}.
        xh8_d = nc.dram_tensor(
            "xh8", [nfp8 * NBLK * 128, 2 * SBLK], dt.float8e4, kind="ExternalInput"
        )
        wq8_d = nc.dram_tensor(
            "wq8", [nfp8 * 128, 2 * ATTN], dt.float8e4, kind="ExternalInput"
        )
        wk8_d = nc.dram_tensor(
            "wk8", [nfp8 * 128, 2 * ATTN], dt.float8e4, kind="ExternalInput"
        )
    if nbh:
        xhb_d = nc.dram_tensor("xhb", [nbh * 128, S], dt.bfloat16, kind="ExternalInput")
        wqb_d = nc.dram_tensor("wqb", [nbh * 128, ATTN], dt.bfloat16, kind="ExternalInput")
        wkb_d = nc.dram_tensor("wkb", [nbh * 128, ATTN], dt.bfloat16, kind="ExternalInput")
    xl_d = nc.dram_tensor("xl", [KL, S], dt.bfloat16, kind="ExternalInput")
    wql_d = nc.dram_tensor("wql", [KL, ATTN], dt.bfloat16, kind="ExternalInput")
    wkl_d = nc.dram_tensor("wkl", [KL, ATTN], dt.bfloat16, kind="ExternalInput")
    out = nc.dram_tensor("out", [128, NCOL], dt.float32, kind="ExternalOutput")

    scale = 1.0 / 32.0
    if scaled:
        scale /= (XSCALE * WSCALE) ** 2

    DR = mybir.MatmulPerfMode.DoubleRow

    with tile.TileContext(nc) as tc:
        with (
            tc.tile_pool(name="wpool", bufs=1) as wpool,
            tc.tile_pool(name="xpool", bufs=2) as xpool,
            tc.tile_pool(name="epool", bufs=2) as epool,
            tc.tile_pool(name="opool", bufs=1) as opool,
            tc.tile_pool(name="psum", bufs=1, space="PSUM") as psum,
        ):
            wq8_sb = wk8_sb = xh8_0 = None
            wqb_sb = wkb_sb = xhb_0 = None
            if nfp8:
                wq8_sb = wpool.tile([128, nfp8, 2, ATTN], dt.float8e4, tag="wq8")
                wk8_sb = wpool.tile([128, nfp8, 2, ATTN], dt.float8e4, tag="wk8")
                xh8_0 = xpool.tile([128, nfp8, 2, SBLK], dt.float8e4, tag="xh8")
            if nbh:
                wqb_sb = wpool.tile([128, nbh, ATTN], dt.bfloat16, tag="wqb")
                wkb_sb = wpool.tile([128, nbh, ATTN], dt.bfloat16, tag="wkb")
                xhb_0 = xpool.tile([128, nbh, SBLK], dt.bfloat16, tag="xhb")

            def dma_x(xh8, xhb, blk):
                for c in range(nfp8):
                    r0 = (c * NBLK + blk) * 128
                    nc.sync.dma_start(xh8[:, c, :, :], xh8_d[r0 : r0 + 128, :])
                c0 = blk * SBLK
                for j in range(nbh):
                    nc.sync.dma_start(
                        xhb[:, j, :], xhb_d[j * 128 : (j + 1) * 128, c0 : c0 + SBLK]
                    )

            # startup: small l/bias tensors first (they gate tile 0's
            # opening l-chunk matmuls), then weight chunks interleaved with
            # block-0 x chunks.
            wql_sb = wpool.tile([KL, ATTN], dt.bfloat16, tag="wql")
            wkl_sb = wpool.tile([KL, ATTN], dt.bfloat16, tag="wkl")
            nc.sync.dma_start(wql_sb[:], wql_d[:])
            nc.sync.dma_start(wkl_sb[:], wkl_d[:])
            xl0 = xpool.tile([KL, SBLK], dt.bfloat16, tag="xl", name="xl0")
            nc.sync.dma_start(xl0[:], xl_d[:, 0:SBLK])
            for c in range(nfp8):
                if c == 0:
                    nc.gpsimd.dma_start(
                        xh8_0[:, 0, :, :], xh8_d[0 : 128, :]
                    )
                    for nh in range(2):
                        n0 = nh * ATTN
                        nc.sync.dma_start(
                            wq8_sb[:, 0, nh, :], wq8_d[0:128, n0 : n0 + ATTN]
                        )
                        nc.sync.dma_start(
                            wk8_sb[:, 0, nh, :], wk8_d[0:128, n0 : n0 + ATTN]
                        )
                else:
                    nc.sync.dma_start(wq8_sb[:, c, :, :], wq8_d[c * 128 : (c + 1) * 128, :])
                    nc.sync.dma_start(wk8_sb[:, c, :, :], wk8_d[c * 128 : (c + 1) * 128, :])
                    r0 = c * NBLK * 128
                    nc.sync.dma_start(xh8_0[:, c, :, :], xh8_d[r0 : r0 + 128, :])
            for j in range(nbh):
                nc.sync.dma_start(wqb_sb[:, j, :], wqb_d[j * 128 : (j + 1) * 128, :])
                nc.sync.dma_start(wkb_sb[:, j, :], wkb_d[j * 128 : (j + 1) * 128, :])
                nc.sync.dma_start(xhb_0[:, j, :], xhb_d[j * 128 : (j + 1) * 128, 0:SBLK])
            score_sb = opool.tile([128, NCOL], dt.float32, tag="score")

            # one accumulation step: all h chunks for one psum target
            def h_chunks(ps, xh8, xhb, wsb8, wsbb, s0):
                n_mm = 0
                for c in range(nfp8):
                    lhs = xh8[:, c, :, s0 : s0 + 128]
                    for nh in range(2):
                        n0 = nh * 512
                        nc.tensor.matmul(
                            ps[:, n0 : n0 + 512], lhs, wsb8[:, c, :, n0 : n0 + 512],
                            start=(n_mm == 0), stop=False, perf_mode=DR,
                        )
                    n_mm += 1
                for j in range(nbh):
                    lhs = xhb[:, j, s0 : s0 + 128]
                    for nh in range(2):
                        n0 = nh * 512
                        nc.tensor.matmul(
                            ps[:, n0 : n0 + 512], lhs, wsbb[:, j, n0 : n0 + 512],
                            start=(n_mm == 0), stop=False,
                        )
                    n_mm += 1

            def l_chunk(ps, wlsb, lhs_l, start=False, stop=True):
                for nh in range(2):
                    n0 = nh * 512
                    nc.tensor.matmul(
                        ps[:, n0 : n0 + 512], lhs_l, wlsb[:, n0 : n0 + 512],
                        start=start, stop=stop,
                    )

            def epilogue(psq, psk, col, split=False):
                if split:
                    # Final tile: q relu runs full-width under the k matmul
                    # pass; k in halves feeding a DVE mult/reduce chain.
                    qsb = epool.tile([128, ATTN], dt.bfloat16, tag="qsb")
                    nc.scalar.activation(
                        qsb[:], psq[:], mybir.ActivationFunctionType.Relu,
                        scale=scale,
                    )
                    sc2 = epool.tile([128, 2], dt.float32, tag="sc2")
                    for nh in range(2):
                        n0 = nh * 512
                        ksh = epool.tile([128, 512], dt.bfloat16, tag="ksh")
                        nc.scalar.activation(
                            ksh[:], psk[:, n0 : n0 + 512],
                            mybir.ActivationFunctionType.Relu,
                        )
                        prh = epool.tile([128, 512], dt.bfloat16, tag="prh")
                        nc.vector.tensor_mul(prh[:], qsb[:, n0 : n0 + 512], ksh[:])
                        nc.vector.tensor_reduce(
                            sc2[:, nh : nh + 1], prh[:],
                            axis=mybir.AxisListType.X, op=mybir.AluOpType.add,
                        )
                    nc.vector.tensor_reduce(
                        score_sb[:, col : col + 1], sc2[:],
                        axis=mybir.AxisListType.X, op=mybir.AluOpType.add,
                    )
                    return
                qsb = epool.tile([128, ATTN], dt.bfloat16, tag="qsb")
                nc.scalar.activation(
                    qsb[:], psq[:], mybir.ActivationFunctionType.Relu
                )
                ksb = epool.tile([128, ATTN], dt.bfloat16, tag="ksb")
                nc.scalar.activation(
                    ksb[:], psk[:], mybir.ActivationFunctionType.Relu
                )
                prod = epool.tile([128, ATTN], dt.bfloat16, tag="prod")
                nc.vector.tensor_mul(prod[:], qsb[:], ksb[:])
                cpy = epool.tile([128, ATTN], dt.bfloat16, tag="cpy")
                nc.scalar.activation(
                    cpy[:],
                    prod[:],
                    mybir.ActivationFunctionType.Copy,
                    scale=scale,
                    accum_out=score_sb[:, col : col + 1],
                )

            for blk in range(NBLK):
                c0 = blk * SBLK
                if blk == 0:
                    xh8 = xh8_0
                    xhb = xhb_0
                    xl = xl0
                else:
                    xh8 = xhb = None
                    if nfp8:
                        xh8 = xpool.tile(
                            [128, nfp8, 2, SBLK], dt.float8e4, tag="xh8",
                            name=f"xh8_{blk}",
                        )
                    if nbh:
                        xhb = xpool.tile(
                            [128, nbh, SBLK], dt.bfloat16, tag="xhb",
                            name=f"xhb_{blk}",
                        )
                    dma_x(xh8, xhb, blk)
                    xl = xpool.tile([KL, SBLK], dt.bfloat16, tag="xl")
                    nc.sync.dma_start(xl[:], xl_d[:, c0 : c0 + SBLK])

                for t in range(NT):
                    is_last = blk == NBLK - 1 and t == NT - 1
                    psq = psum.tile(
                        [128, ATTN], dt.float32, tag="psq", bufs=2,
                        name=f"psq_{blk}_{t}",
                    )
                    psk = psum.tile(
                        [128, ATTN], dt.float32, tag="psk", bufs=2,
                        name=f"psk_{blk}_{t}",
                    )
                    s0 = t * 128
                    lhs_l = xl[:, s0 : s0 + 128]
                    if not is_last:
                        # l/bias chunk FIRST (needs only the small tensors,
                        # so tile 0 can start before the big weight chunks
                        # land and absorb the PE cold-clock ramp), then the
                        # fp8 and bf16 h chunks; accumulation is commutative.
                        l_chunk(psq, wql_sb, lhs_l, start=True, stop=False)
                        l_chunk(psk, wkl_sb, lhs_l, start=True, stop=False)
                        for c in range(nfp8):
                            lhs = xh8[:, c, :, s0 : s0 + 128]
                            stop8 = nbh == 0 and c == nfp8 - 1
                            for nh in range(2):
                                n0 = nh * 512
                                nc.tensor.matmul(
                                    psq[:, n0 : n0 + 512], lhs,
                                    wq8_sb[:, c, :, n0 : n0 + 512],
                                    start=False, stop=stop8,
                                    perf_mode=DR,
                                )
                                nc.tensor.matmul(
                                    psk[:, n0 : n0 + 512], lhs,
                                    wk8_sb[:, c, :, n0 : n0 + 512],
                                    start=False, stop=stop8,
                                    perf_mode=DR,
                                )
                        for j in range(nbh):
                            lhs = xhb[:, j, s0 : s0 + 128]
                            for nh in range(2):
                                n0 = nh * 512
                                nc.tensor.matmul(
                                    psq[:, n0 : n0 + 512], lhs,
                                    wqb_sb[:, j, n0 : n0 + 512],
                                    start=False, stop=(j == nbh - 1),
                                )
                                nc.tensor.matmul(
                                    psk[:, n0 : n0 + 512], lhs,
                                    wkb_sb[:, j, n0 : n0 + 512],
                                    start=False, stop=(j == nbh - 1),
                                )
                    else:
                        # last tile: q pass fully before k pass
                        h_chunks(psq, xh8, xhb, wq8_sb, wqb_sb, s0)
                        l_chunk(psq, wql_sb, lhs_l)
                        h_chunks(psk, xh8, xhb, wk8_sb, wkb_sb, s0)
                        l_chunk(psk, wkl_sb, lhs_l)
                    epilogue(psq, psk, blk * NT + t, split=is_last)

            nc.sync.dma_start(out[:], score_sb[:])

    nc.compile()
    return nc


def _get_nc():
    if "nc" not in _CACHE:
        _CACHE["nc"] = _build_nc()
    return _CACHE["nc"]


def prep_in_maps(h, mask, g, l, Wq, bq, Wk, bk, Wv=None, bv=None, nfp8=NFP8):
    import concourse.mybir as mybir

    FP8 = mybir.dt.np(mybir.dt.float8e4)
    nbh = (H - 256 * nfp8) // 128
    h8 = 256 * nfp8                      # h dims handled in fp8

    h = np.asarray(h, dtype=np.float32)
    g = np.asarray(g, dtype=np.float32)
    l_ = np.asarray(l, dtype=np.float32)
    Wq = np.asarray(Wq, dtype=np.float32)
    bq = np.asarray(bq, dtype=np.float32)
    Wk = np.asarray(Wk, dtype=np.float32)
    bk = np.asarray(bk, dtype=np.float32)

    # Fold the per-batch g contribution into the bias (fp32 on host).
    bq_eff = bq[None, :] + g @ Wq[H : H + LOC]          # [B, ATTN]
    bk_eff = bk[None, :] + g @ Wk[H : H + LOC]

    xs = XSCALE if nfp8 else 1.0
    ws = WSCALE if nfp8 else 1.0

    base = {}
    if nfp8:
        base["wq8"] = np.ascontiguousarray((Wq[:h8] * ws).astype(FP8)).reshape(
            nfp8 * 128, 2 * ATTN
        )
        base["wk8"] = np.ascontiguousarray((Wk[:h8] * ws).astype(FP8)).reshape(
            nfp8 * 128, 2 * ATTN
        )
    if nbh:
        base["wqb"] = (Wq[h8:H] * ws).astype(BF16)
        base["wkb"] = (Wk[h8:H] * ws).astype(BF16)

    wql = np.empty((KL, ATTN), dtype=BF16)
    wql[:LOC] = Wq[H + LOC :] * ws
    wkl = np.empty((KL, ATTN), dtype=BF16)
    wkl[:LOC] = Wk[H + LOC :] * ws

    in_maps = []
    for b in range(B):
        m = dict(base)
        hT = h[b].T
        if nfp8:
            x8 = (hT[:h8] * xs).astype(FP8)
            m["xh8"] = np.ascontiguousarray(
                x8.reshape(nfp8, 128, 2, NBLK, SBLK).transpose(0, 3, 1, 2, 4)
            ).reshape(nfp8 * NBLK * 128, 2 * SBLK)
        if nbh:
            m["xhb"] = np.ascontiguousarray(hT[h8:] * xs).astype(BF16)
        xl = np.empty((KL, S), dtype=BF16)
        xl[:LOC] = l_[b].T * xs
        xl[LOC] = xs
        m["xl"] = xl
        # ones-row carries xs, so the bias row needs only ws.
        wql_b = wql.copy()
        wql_b[LOC] = bq_eff[b] * ws
        wkl_b = wkl.copy()
        wkl_b[LOC] = bk_eff[b] * ws
        m["wql"] = wql_b
        m["wkl"] = wkl_b
        in_maps.append(m)
    return in_maps


def kernel(h, mask, g, l, Wq, bq, Wk, bk, Wv=None, bv=None):
    from concourse.bass_utils import run_bass_kernel_spmd

    mask = np.asarray(mask)
    in_maps = prep_in_maps(h, mask, g, l, Wq, bq, Wk, bk)

    nc = _get_nc()
    res = run_bass_kernel_spmd(nc, in_maps, core_ids=list(range(B)), trace=False)

    scores = np.empty((B, S), dtype=np.float32)
    for b in range(B):
        scores[b] = res.results[b]["out"].T.reshape(S)
    return np.where(mask == 1, np.float32(-1e9), scores).astype(np.float32)


# revision 35
# speedup vs baseline: 1.0048x; 1.0048x over previous
"""Trainium2 Bass kernel for nn_MultiHeadAttention_89232240541956.

Computes, for B=8, S=4096, H=1024, ATTN=1024, EXT=1152:
    x_ext = [h | broadcast(g) | l]                       [B, S, 1152]
    q = relu(x_ext @ Wq + bq); k = relu(x_ext @ Wk + bk) [B, S, 1024]
    scores = sum(q * k, -1) / 32, masked to -1e9 where mask == 1

Sharding: data-parallel over batch — core b owns batch b.

Key transformations:
  - v (Wv, bv) is dead code in the reference's early-return path — skipped.
  - g @ Wq[1024:1088] is constant over seq for a batch — folded into the
    bias on the host, so the device contracts over 1024 (h) + 64 (l) only.
  - Bias folded into the matmul as one extra contraction row (ones-row in
    x^T against a bias-row in W).
  - Host pre-transposes to x^T so the contraction dim lands on SBUF
    partitions with no on-device transposes; all DMAs are 2-D contiguous.
  - Mixed precision: NFP8 paired chunks (256 contraction dims each) run in
    fp8 E4M3 with DoubleRow (2 MACs/cell/cycle, halves the matmul count
    for those dims); the rest in bf16. x is pre-scaled by 16 and W by 64
    (lossless exponent shifts) to sit in E4M3's normal range; the 2^10
    product scale and 1/sqrt(1024) are divided out in the epilogue.
    Measured on the actual (seed-0) data vs the fp32 reference:
    NFP8=0: 1.7e-3 max rel err @ ~273us; NFP8=2: 1.6e-2 @ ~217us (ships);
    NFP8=4: 2.2e-2 @ ~163us. Max abs err at NFP8=2 is 0.025 against a
    score scale of ~1.7 mean / ~4.5 absmax (masked entries exact).
  - Epilogue: ScalarE relu (PSUM->SBUF bf16), DVE multiply, ScalarE
    Copy-with-accum row reduction; masking on host at unshard. The last
    tile runs q/k as separate matmul passes with a split-half epilogue to
    shorten the kernel tail.
  - PSUM: q/k tiles 2 banks each, double-buffered = all 8 banks; N=512
    per matmul (one fp32 bank) with accumulating chunk groups.
"""

import numpy as np
import ml_dtypes

B, S, H, LOC = 8, 4096, 1024, 64
ATTN = 1024
KL = LOC + 1              # 65 rows: l | ones/bias
SBLK = 512                # seq columns per DMA block
NBLK = S // SBLK          # 8
NT = SBLK // 128          # 4 seq tiles (128 tokens) per block
NCOL = S // 128           # 32 output columns

BF16 = ml_dtypes.bfloat16

# Number of paired fp8 chunks (256 h-dims each); remaining h-dims in bf16.
NFP8 = 2
XSCALE = 16.0
WSCALE = 64.0

_CACHE = {}


def _build_nc(nfp8=NFP8):
    import concourse.bass as bass
    import concourse.mybir as mybir
    import concourse.tile as tile
    from concourse import bacc

    dt = mybir.dt
    nbh = (H - 256 * nfp8) // 128         # bf16 h chunks
    scaled = nfp8 > 0
    nc = bacc.Bacc(None, target_bir_lowering=False)

    xh8_d = wq8_d = wk8_d = None
    xhb_d = wqb_d = wkb_d = None
    if nfp8:
        # paired layout: [c, blk, p, j, s] rows flattened; partition p of
        # chunk c holds contraction rows c*256 + 2p + j for j in {0,1}.
        xh8_d = nc.dram_tensor(
            "xh8", [nfp8 * NBLK * 128, 2 * SBLK], dt.float8e4, kind="ExternalInput"
        )
        wq8_d = nc.dram_tensor(
            "wq8", [nfp8 * 128, 2 * ATTN], dt.float8e4, kind="ExternalInput"
        )
        wk8_d = nc.dram_tensor(
            "wk8", [nfp8 * 128, 2 * ATTN], dt.float8e4, kind="ExternalInput"
        )
    if nbh:
        xhb_d = nc.dram_tensor("xhb", [nbh * 128, S], dt.bfloat16, kind="ExternalInput")
        wqb_d = nc.dram_tensor("wqb", [nbh * 128, ATTN], dt.bfloat16, kind="ExternalInput")
        wkb_d = nc.dram_tensor("wkb", [nbh * 128, ATTN], dt.bfloat16, kind="ExternalInput")
    xl_d = nc.dram_tensor("xl", [KL, S], dt.bfloat16, kind="ExternalInput")
    wql_d = nc.dram_tensor("wql", [KL, ATTN], dt.bfloat16, kind="ExternalInput")
    wkl_d = nc.dram_tensor("wkl", [KL, ATTN], dt.bfloat16, kind="ExternalInput")
    out = nc.dram_tensor("out", [128, NCOL], dt.float32, kind="ExternalOutput")

    scale = 1.0 / 32.0
    if scaled:
        scale /= (XSCALE * WSCALE) ** 2

    DR = mybir.MatmulPerfMode.DoubleRow

    with tile.TileContext(nc) as tc:
        with (
            tc.tile_pool(name="wpool", bufs=1) as wpool,
            tc.tile_pool(name="xpool", bufs=2) as xpool,
            tc.tile_pool(name="epool", bufs=2) as epool,
            tc.tile_pool(name="opool", bufs=1) as opool,
            tc.tile_pool(name="psum", bufs=1, space="PSUM") as psum,
        ):
            wq8_sb = wk8_sb = xh8_0 = None
            wqb_sb = wkb_sb = xhb_0 = None
            if nfp8:
                wq8_sb = wpool.tile([128, nfp8, 2, ATTN], dt.float8e4, tag="wq8")
                wk8_sb = wpool.tile([128, nfp8, 2, ATTN], dt.float8e4, tag="wk8")
                xh8_0 = xpool.tile([128, nfp8, 2, SBLK], dt.float8e4, tag="xh8")
            if nbh:
                wqb_sb = wpool.tile([128, nbh, ATTN], dt.bfloat16, tag="wqb")
                wkb_sb = wpool.tile([128, nbh, ATTN], dt.bfloat16, tag="wkb")
                xhb_0 = xpool.tile([128, nbh, SBLK], dt.bfloat16, tag="xhb")

            def dma_x(xh8, xhb, blk):
                for c in range(nfp8):
                    r0 = (c * NBLK + blk) * 128
                    nc.sync.dma_start(xh8[:, c, :, :], xh8_d[r0 : r0 + 128, :])
                c0 = blk * SBLK
                for j in range(nbh):
                    nc.sync.dma_start(
                        xhb[:, j, :], xhb_d[j * 128 : (j + 1) * 128, c0 : c0 + SBLK]
                    )

            # startup: weight chunks interleaved with block-0 x chunks.
            wql_sb = wpool.tile([KL, ATTN], dt.bfloat16, tag="wql")
            wkl_sb = wpool.tile([KL, ATTN], dt.bfloat16, tag="wkl")
            for c in range(nfp8):
                if c == 0:
                    nc.gpsimd.dma_start(
                        xh8_0[:, 0, :, :], xh8_d[0 : 128, :]
                    )
                    for nh in range(2):
                        n0 = nh * ATTN
                        nc.sync.dma_start(
                            wq8_sb[:, 0, nh, :], wq8_d[0:128, n0 : n0 + ATTN]
                        )
                        nc.sync.dma_start(
                            wk8_sb[:, 0, nh, :], wk8_d[0:128, n0 : n0 + ATTN]
                        )
                else:
                    nc.sync.dma_start(wq8_sb[:, c, :, :], wq8_d[c * 128 : (c + 1) * 128, :])
                    nc.sync.dma_start(wk8_sb[:, c, :, :], wk8_d[c * 128 : (c + 1) * 128, :])
                    r0 = c * NBLK * 128
                    nc.sync.dma_start(xh8_0[:, c, :, :], xh8_d[r0 : r0 + 128, :])
            for j in range(nbh):
                nc.sync.dma_start(wqb_sb[:, j, :], wqb_d[j * 128 : (j + 1) * 128, :])
                nc.sync.dma_start(wkb_sb[:, j, :], wkb_d[j * 128 : (j + 1) * 128, :])
                nc.sync.dma_start(xhb_0[:, j, :], xhb_d[j * 128 : (j + 1) * 128, 0:SBLK])
            nc.sync.dma_start(wql_sb[:], wql_d[:])
            nc.sync.dma_start(wkl_sb[:], wkl_d[:])
            xl0 = xpool.tile([KL, SBLK], dt.bfloat16, tag="xl", name="xl0")
            nc.sync.dma_start(xl0[:], xl_d[:, 0:SBLK])

            score_sb = opool.tile([128, NCOL], dt.float32, tag="score")

            # one accumulation step: all h chunks for one psum target
            def h_chunks(ps, xh8, xhb, wsb8, wsbb, s0):
                n_mm = 0
                for c in range(nfp8):
                    lhs = xh8[:, c, :, s0 : s0 + 128]
                    for nh in range(2):
                        n0 = nh * 512
                        nc.tensor.matmul(
                            ps[:, n0 : n0 + 512], lhs, wsb8[:, c, :, n0 : n0 + 512],
                            start=(n_mm == 0), stop=False, perf_mode=DR,
                        )
                    n_mm += 1
                for j in range(nbh):
                    lhs = xhb[:, j, s0 : s0 + 128]
                    for nh in range(2):
                        n0 = nh * 512
                        nc.tensor.matmul(
                            ps[:, n0 : n0 + 512], lhs, wsbb[:, j, n0 : n0 + 512],
                            start=(n_mm == 0), stop=False,
                        )
                    n_mm += 1

            def l_chunk(ps, wlsb, lhs_l, start=False, stop=True):
                for nh in range(2):
                    n0 = nh * 512
                    nc.tensor.matmul(
                        ps[:, n0 : n0 + 512], lhs_l, wlsb[:, n0 : n0 + 512],
                        start=start, stop=stop,
                    )

            def epilogue(psq, psk, col, split=False):
                if split:
                    # Final tile: q relu runs full-width under the k matmul
                    # pass; k in halves feeding a DVE mult/reduce chain.
                    qsb = epool.tile([128, ATTN], dt.bfloat16, tag="qsb")
                    nc.scalar.activation(
                        qsb[:], psq[:], mybir.ActivationFunctionType.Relu,
                        scale=scale,
                    )
                    sc2 = epool.tile([128, 2], dt.float32, tag="sc2")
                    for nh in range(2):
                        n0 = nh * 512
                        ksh = epool.tile([128, 512], dt.bfloat16, tag="ksh")
                        nc.scalar.activation(
                            ksh[:], psk[:, n0 : n0 + 512],
                            mybir.ActivationFunctionType.Relu,
                        )
                        prh = epool.tile([128, 512], dt.bfloat16, tag="prh")
                        nc.vector.tensor_mul(prh[:], qsb[:, n0 : n0 + 512], ksh[:])
                        nc.vector.tensor_reduce(
                            sc2[:, nh : nh + 1], prh[:],
                            axis=mybir.AxisListType.X, op=mybir.AluOpType.add,
                        )
                    nc.vector.tensor_reduce(
                        score_sb[:, col : col + 1], sc2[:],
                        axis=mybir.AxisListType.X, op=mybir.AluOpType.add,
                    )
                    return
                qsb = epool.tile([128, ATTN], dt.bfloat16, tag="qsb")
                nc.scalar.activation(
                    qsb[:], psq[:], mybir.ActivationFunctionType.Relu
                )
                ksb = epool.tile([128, ATTN], dt.bfloat16, tag="ksb")
                nc.scalar.activation(
                    ksb[:], psk[:], mybir.ActivationFunctionType.Relu
                )
                prod = epool.tile([128, ATTN], dt.bfloat16, tag="prod")
                nc.vector.tensor_mul(prod[:], qsb[:], ksb[:])
                cpy = epool.tile([128, ATTN], dt.bfloat16, tag="cpy")
                nc.scalar.activation(
                    cpy[:],
                    prod[:],
                    mybir.ActivationFunctionType.Copy,
                    scale=scale,
                    accum_out=score_sb[:, col : col + 1],
                )

            for blk in range(NBLK):
                c0 = blk * SBLK
                if blk == 0:
                    xh8 = xh8_0
                    xhb = xhb_0
                    xl = xl0
                else:
                    xh8 = xhb = None
                    if nfp8:
                        xh8 = xpool.tile(
                            [128, nfp8, 2, SBLK], dt.float8e4, tag="xh8",
                            name=f"xh8_{blk}",
                        )
                    if nbh:
                        xhb = xpool.tile(
                            [128, nbh, SBLK], dt.bfloat16, tag="xhb",
                            name=f"xhb_{blk}",
                        )
                    dma_x(xh8, xhb, blk)
                    xl = xpool.tile([KL, SBLK], dt.bfloat16, tag="xl")
                    nc.sync.dma_start(xl[:], xl_d[:, c0 : c0 + SBLK])

                for t in range(NT):
                    is_last = blk == NBLK - 1 and t == NT - 1
                    psq = psum.tile(
                        [128, ATTN], dt.float32, tag="psq", bufs=2,
                        name=f"psq_{blk}_{t}",
                    )
                    psk = psum.tile(
                        [128, ATTN], dt.float32, tag="psk", bufs=2,
                        name=f"psk_{blk}_{t}",
                    )
                    s0 = t * 128
                    lhs_l = xl[:, s0 : s0 + 128]
                    if not is_last:
                        # interleave q/k per chunk for stationary reuse
                        for c in range(nfp8):
                            lhs = xh8[:, c, :, s0 : s0 + 128]
                            for nh in range(2):
                                n0 = nh * 512
                                nc.tensor.matmul(
                                    psq[:, n0 : n0 + 512], lhs,
                                    wq8_sb[:, c, :, n0 : n0 + 512],
                                    start=(c == 0), stop=False,
                                    perf_mode=DR,
                                )
                                nc.tensor.matmul(
                                    psk[:, n0 : n0 + 512], lhs,
                                    wk8_sb[:, c, :, n0 : n0 + 512],
                                    start=(c == 0), stop=False,
                                    perf_mode=DR,
                                )
                        for j in range(nbh):
                            lhs = xhb[:, j, s0 : s0 + 128]
                            for nh in range(2):
                                n0 = nh * 512
                                nc.tensor.matmul(
                                    psq[:, n0 : n0 + 512], lhs,
                                    wqb_sb[:, j, n0 : n0 + 512],
                                    start=(nfp8 == 0 and j == 0),
                                    stop=False,
                                )
                                nc.tensor.matmul(
                                    psk[:, n0 : n0 + 512], lhs,
                                    wkb_sb[:, j, n0 : n0 + 512],
                                    start=(nfp8 == 0 and j == 0),
                                    stop=False,
                                )
                        l_chunk(psq, wql_sb, lhs_l)
                        l_chunk(psk, wkl_sb, lhs_l)
                    else:
                        # last tile: q pass fully before k pass
                        h_chunks(psq, xh8, xhb, wq8_sb, wqb_sb, s0)
                        l_chunk(psq, wql_sb, lhs_l)
                        h_chunks(psk, xh8, xhb, wk8_sb, wkb_sb, s0)
                        l_chunk(psk, wkl_sb, lhs_l)
                    epilogue(psq, psk, blk * NT + t, split=is_last)

            nc.sync.dma_start(out[:], score_sb[:])

    nc.compile()
    return nc


def _get_nc():
    if "nc" not in _CACHE:
        _CACHE["nc"] = _build_nc()
    return _CACHE["nc"]


def prep_in_maps(h, mask, g, l, Wq, bq, Wk, bk, Wv=None, bv=None, nfp8=NFP8):
    import concourse.mybir as mybir

    FP8 = mybir.dt.np(mybir.dt.float8e4)
    nbh = (H - 256 * nfp8) // 128
    h8 = 256 * nfp8                      # h dims handled in fp8

    h = np.asarray(h, dtype=np.float32)
    g = np.asarray(g, dtype=np.float32)
    l_ = np.asarray(l, dtype=np.float32)
    Wq = np.asarray(Wq, dtype=np.float32)
    bq = np.asarray(bq, dtype=np.float32)
    Wk = np.asarray(Wk, dtype=np.float32)
    bk = np.asarray(bk, dtype=np.float32)

    # Fold the per-batch g contribution into the bias (fp32 on host).
    bq_eff = bq[None, :] + g @ Wq[H : H + LOC]          # [B, ATTN]
    bk_eff = bk[None, :] + g @ Wk[H : H + LOC]

    xs = XSCALE if nfp8 else 1.0
    ws = WSCALE if nfp8 else 1.0

    base = {}
    if nfp8:
        base["wq8"] = np.ascontiguousarray((Wq[:h8] * ws).astype(FP8)).reshape(
            nfp8 * 128, 2 * ATTN
        )
        base["wk8"] = np.ascontiguousarray((Wk[:h8] * ws).astype(FP8)).reshape(
            nfp8 * 128, 2 * ATTN
        )
    if nbh:
        base["wqb"] = (Wq[h8:H] * ws).astype(BF16)
        base["wkb"] = (Wk[h8:H] * ws).astype(BF16)

    wql = np.empty((KL, ATTN), dtype=BF16)
    wql[:LOC] = Wq[H + LOC :] * ws
    wkl = np.empty((KL, ATTN), dtype=BF16)
    wkl[:LOC] = Wk[H + LOC :] * ws

    in_maps = []
    for b in range(B):
        m = dict(base)
        hT = h[b].T
        if nfp8:
            x8 = (hT[:h8] * xs).astype(FP8)
            m["xh8"] = np.ascontiguousarray(
                x8.reshape(nfp8, 128, 2, NBLK, SBLK).transpose(0, 3, 1, 2, 4)
            ).reshape(nfp8 * NBLK * 128, 2 * SBLK)
        if nbh:
            m["xhb"] = np.ascontiguousarray(hT[h8:] * xs).astype(BF16)
        xl = np.empty((KL, S), dtype=BF16)
        xl[:LOC] = l_[b].T * xs
        xl[LOC] = xs
        m["xl"] = xl
        # ones-row carries xs, so the bias row needs only ws.
        wql_b = wql.copy()
        wql_b[LOC] = bq_eff[b] * ws
        wkl_b = wkl.copy()
        wkl_b[LOC] = bk_eff[b] * ws
        m["wql"] = wql_b
        m["wkl"] = wkl_b
        in_maps.append(m)
    return in_maps


def kernel(h, mask, g, l, Wq, bq, Wk, bk, Wv=None, bv=None):
    from concourse.bass_utils import run_bass_kernel_spmd

    mask = np.asarray(mask)
    in_maps = prep_in_maps(h, mask, g, l, Wq, bq, Wk, bk)

    nc = _get_nc()
    res = run_bass_kernel_spmd(nc, in_maps, core_ids=list(range(B)), trace=False)

    scores = np.empty((B, S), dtype=np.float32)
    for b in range(B):
        scores[b] = res.results[b]["out"].T.reshape(S)
    return np.where(mask == 1, np.float32(-1e9), scores).astype(np.float32)
